# revision 13
# baseline (speedup 1.0000x reference)
"""Trainium2 Bass kernel for causal self-attention (B=4, T=2048, C=2048, H=16).

Sharding: 8 cores = 4 batches x 2 head-groups (8 heads each).
v2 architecture (all-bf16 dataflow, fused QK-projection + SDPA pipeline):
  A) v = x @ Wv (bf16)            -> DRAM spill [T, 1024] bf16
  B+C fused, per head h (software pipelined, slot h = C(h) || B(h+1)):
     B-step: qT/kT = RoPE(Wqk^T x^T + b)  -> SBUF resident bf16 [128, T]
     C-step: per (t, j): scores matmul -> exp (ACT, bf16) -> tri mask ->
             PV matmul + denominator matmul (ones lhsT) accumulate in PSUM;
             per t: rden = reciprocal_approx_fast(den); y = psy * rden (bf16)
  D) partial_out = y^T @ wp  -> [T, C] fp32 partial
Host sums core pairs per batch, adds b_proj and the folded v-bias term
(bv @ wp) per core.
"""

import sys

import numpy as np

sys.path.insert(0, "/opt/trn_rl_repo")

import ml_dtypes  # noqa: E402

import concourse.bass as bass  # noqa: E402,F401
import concourse.mybir as mybir  # noqa: E402
import concourse.tile as tile  # noqa: E402
from concourse import bacc  # noqa: E402

F32 = mybir.dt.float32
BF16 = mybir.dt.bfloat16
AF = mybir.ActivationFunctionType
BF_NP = ml_dtypes.bfloat16

B, T, C = 4, 2048, 2048
H, D = 16, 128
HPC = 8            # heads per core
P = 128
NT = 512           # matmul moving free dim
TT = T // NT       # 4 token tiles
CC = C // P        # 16 contraction chunks over C
HD = D // 2        # 64 (rope half)
ROPE_BASE = 10000.0
LAG = 3            # score->PV software pipeline depth (in j blocks)

_CACHE = {}


def build_program():
    nc = bacc.Bacc(name="csa_tp2")

    xt = nc.dram_tensor("xt", (C, T), BF16, kind="ExternalInput")
    wv = nc.dram_tensor("wv", (C, HPC * D), BF16, kind="ExternalInput")
    # per-head blocks of [C, 2P] (q columns scaled, then k columns)
    wqk = nc.dram_tensor("wqk", (HPC * C, 2 * P), BF16, kind="ExternalInput")
    bqk = nc.dram_tensor("bqk", (P, 2 * HPC), F32, kind="ExternalInput")
    cs = nc.dram_tensor("cs", (P, T), BF16, kind="ExternalInput")
    sw = nc.dram_tensor("sw", (P, T), BF16, kind="ExternalInput")
    tri = nc.dram_tensor("tri", (P, P), BF16, kind="ExternalInput")
    onesf = nc.dram_tensor("onesf", (P, P), mybir.dt.float32r,
                           kind="ExternalInput")
    wp = nc.dram_tensor("wp", (HPC * D, C), BF16, kind="ExternalInput")
    out = nc.dram_tensor("out", (T, C), F32, kind="ExternalOutput")

    v_spill = nc.dram_tensor("v_spill", (T, HPC * D), BF16, kind="Internal")

    def _mm(o, lhsT, rhs, **kw):
        nc.tensor.matmul(o, lhsT, rhs, **kw)

    with tile.TileContext(nc) as tc:
        with (
            tc.tile_pool(name="tabs", bufs=1) as tabs,
            tc.tile_pool(name="work", bufs=1) as work,
            tc.tile_pool(name="ypool", bufs=1) as ypool,
            tc.tile_pool(name="xtp", bufs=1) as xtp,
        ):
            # ---- tables + first wqk prefetches (gpsimd queue) ----
            cs_t = tabs.tile([P, T], BF16, tag="cs", name="cs")
            nc.gpsimd.dma_start(cs_t[:], cs[:])
            sw_t = tabs.tile([P, T], BF16, tag="sw", name="sw")
            nc.gpsimd.dma_start(sw_t[:], sw[:])
            tri_t = tabs.tile([P, P], BF16, tag="tri", name="tri")
            nc.gpsimd.dma_start(tri_t[:], tri[:])
            ones_t = tabs.tile([P, P], mybir.dt.float32r, tag="ones",
                               name="ones")
            nc.gpsimd.dma_start(ones_t[:], onesf[:])
            bqk_t = tabs.tile([P, 2 * HPC], F32, tag="bqk", name="bqk")
            nc.gpsimd.dma_start(bqk_t[:], bqk[:])

            wq_ref = {}

            def prefetch_wqk(h):
                w_ = work.tile([P, CC, 2 * P], BF16, tag=f"wq{h % 2}",
                               name=f"wq{h % 2}")
                nc.gpsimd.dma_start(
                    w_[:],
                    wqk[h * C:(h + 1) * C, :].rearrange(
                        "(c p) f -> p c f", p=P),
                )
                wq_ref[h] = w_

            vh_ref = {}

            def prefetch_vh(h):
                v_ = work.tile([P, CC, P], BF16, tag=f"vh{h % 2}",
                               name=f"vh{h % 2}")
                nc.sync.dma_start(
                    v_[:],
                    v_spill[:, h * D:(h + 1) * D].rearrange(
                        "(j p) d -> p j d", p=P),
                )
                vh_ref[h] = v_

            prefetch_wqk(0)
            prefetch_wqk(1)

            # ---- phase A: V projection (bf16), v -> DRAM spill ----
            xtt = [[None] * TT for _ in range(CC)]
            wvh = [[None] * CC for _ in range(2)]
            with (
                tc.tile_pool(name="wvp", bufs=1) as wvp,
                tc.tile_pool(name="psa", bufs=1, space="PSUM") as psa,
            ):
                # interleave first xt token-slice with wv half 0 so the
                # first matmul chain's deps land early (different queues)
                for c in range(CC):
                    x_ = xtp.tile([P, NT], BF16, tag=f"x{c}_0",
                                  name=f"x{c}_0")
                    nc.sync.dma_start(x_[:], xt[c * P:(c + 1) * P, 0:NT])
                    xtt[c][0] = x_
                    w_ = wvp.tile([P, NT], BF16, tag=f"wv0_{c}",
                                  name=f"wv0_{c}")
                    nc.scalar.dma_start(w_[:], wv[c * P:(c + 1) * P, 0:NT])
                    wvh[0][c] = w_
                for t in range(1, TT):
                    for c in range(CC):
                        x_ = xtp.tile([P, NT], BF16, tag=f"x{c}_{t}",
                                      name=f"x{c}_{t}")
                        nc.sync.dma_start(
                            x_[:], xt[c * P:(c + 1) * P, t * NT:(t + 1) * NT])
                        xtt[c][t] = x_
                for c in range(CC):
                    w_ = wvp.tile([P, NT], BF16, tag=f"wv1_{c}",
                                  name=f"wv1_{c}")
                    nc.scalar.dma_start(w_[:], wv[c * P:(c + 1) * P,
                                                  NT:2 * NT])
                    wvh[1][c] = w_

                for n in range(2):
                    for t in range(TT):
                        for m in range(4):
                            mtok = 4 * t + m
                            msl = slice(m * P, (m + 1) * P)
                            ps = psa.tile([P, NT], F32, tag="psa", bufs=8,
                                          name="psa")
                            for c in range(CC):
                                _mm(ps[:], xtt[c][t][:, msl], wvh[n][c][:],
                                    start=(c == 0), stop=(c == CC - 1))
                            vt = work.tile([P, NT], BF16, tag="vt", bufs=2,
                                           name="vt")
                            nc.scalar.copy(vt[:], ps[:])
                            nc.gpsimd.dma_start(
                                v_spill[mtok * P:(mtok + 1) * P,
                                        n * NT:(n + 1) * NT],
                                vt[:],
                            )

            prefetch_vh(0)
            prefetch_vh(1)

            # ---- fused B+C pipeline ----
            with tc.tile_pool(name="wpp", bufs=1) as wpp:
                with tc.tile_pool(name="psbc", bufs=1, space="PSUM") as psbc:
                    qs_ref = {}
                    ks_ref = {}

                    def emit_B_t(h, t):
                        tsl = slice(t * NT, (t + 1) * NT)
                        wq_h = wq_ref[h]
                        for f in range(2):
                            pst = psbc.tile([P, NT], F32, tag="pst", bufs=2,
                                            name="pst")
                            for c in range(CC):
                                _mm(pst[:], wq_h[:, c, f * P:(f + 1) * P],
                                    xtt[c][t][:],
                                    start=(c == 0), stop=(c == CC - 1))
                            bcol = bqk_t[:, 2 * h + f:2 * h + f + 1]
                            rr = work.tile([P, NT], BF16, tag="rr", bufs=2,
                                           name="rr")
                            nc.scalar.activation(
                                rr[:], pst[:], AF.Identity, bias=bcol)
                            rw = work.tile([P, NT], BF16, tag="rw", bufs=2,
                                           name="rw")
                            nc.scalar.activation(
                                rw[0:HD, :], pst[HD:P, :], AF.Identity,
                                bias=bqk_t[HD:P, 2 * h + f:2 * h + f + 1])
                            nc.scalar.activation(
                                rw[HD:P, :], pst[0:HD, :], AF.Identity,
                                bias=bqk_t[0:HD, 2 * h + f:2 * h + f + 1])
                            dst = (qs_ref if f == 0 else ks_ref)[h][:, tsl]
                            tmp = work.tile([P, NT], BF16, tag="tmp", bufs=2,
                                            name="tmp")
                            nc.vector.tensor_mul(dst, rr[:], cs_t[:, tsl])
                            nc.vector.tensor_mul(tmp[:], rw[:], sw_t[:, tsl])
                            nc.vector.tensor_add(dst, dst, tmp[:])

                    def new_qk(h):
                        qs_ref[h] = work.tile([P, T], BF16, tag=f"qs{h % 2}",
                                              name=f"qs{h % 2}")
                        ks_ref[h] = work.tile([P, T], BF16, tag=f"ks{h % 2}",
                                              name=f"ks{h % 2}")

                    def emit_C_t(h, t, y_t):
                        tsl = slice(t * NT, (t + 1) * NT)
                        qs, ks, vh_ = qs_ref[h], ks_ref[h], vh_ref[h]
                        njs = 4 * t + 4
                        psy = psbc.tile([P, NT], F32, tag="psy", bufs=1,
                                        name="psy")
                        p_sum = work.tile([P, NT], mybir.dt.float32r,
                                          tag="p_sum", bufs=2, name="p_sum")
                        saved = [None] * njs
                        for jj in range(njs + LAG):
                            if jj < njs:
                                j = jj
                                off = (j - 4 * t) * P if j >= 4 * t else 0
                                pss = psbc.tile([P, NT], F32, tag="pss",
                                                bufs=4, name="pss")
                                _mm(pss[:, off:], ks[:, j * P:(j + 1) * P],
                                    qs[:, t * NT + off:(t + 1) * NT],
                                    start=True, stop=True)
                                p = work.tile([P, NT], BF16, tag="p", bufs=4,
                                              name="p")
                                nc.scalar.activation(
                                    p[:, off:], pss[:, off:], AF.Exp)
                                if j >= 4 * t:
                                    nc.vector.tensor_mul(
                                        p[:, off:off + P],
                                        p[:, off:off + P], tri_t[:])
                                # denominator partial sums on the (idle)
                                # gpsimd engine, in fp32, SBUF only
                                if j == 0:
                                    nc.gpsimd.tensor_copy(p_sum[:], p[:])
                                else:
                                    nc.gpsimd.tensor_add(
                                        p_sum[:, off:], p_sum[:, off:],
                                        p[:, off:])
                                saved[j] = (p, off)
                            if jj >= LAG:
                                j2 = jj - LAG
                                p2, off2 = saved[j2]
                                _mm(psy[:, off2:], vh_[:, j2, :],
                                    p2[:, off2:],
                                    start=(j2 == 0), stop=(j2 == njs - 1))
                        den = psbc.tile([P, NT], F32, tag="psd", bufs=1,
                                        name="psd")
                        _mm(den[:], ones_t[:], p_sum[:],
                            start=True, stop=True)
                        rden = work.tile([P, NT], F32, tag="rden", bufs=2,
                                         name="rden")
                        nc.vector.reciprocal_approx_fast(
                            out=rden[:], in_=den[:])
                        nc.vector.tensor_mul(y_t[:, tsl], psy[:], rden[:])

                    # prologue: B(0)
                    new_qk(0)
                    for t in range(TT):
                        emit_B_t(0, t)

                    y_ref = {}
                    wp_t = []
                    for h in range(HPC):
                        y_ref[h] = ypool.tile([P, T], BF16, tag=f"y{h}",
                                              name=f"y{h}")
                        if h + 2 < HPC:
                            prefetch_wqk(h + 2)
                            prefetch_vh(h + 2)
                        if h + 1 < HPC:
                            new_qk(h + 1)
                        if h == HPC - 1:
                            # wp loads (vector queue) ahead of phase D
                            for hh in range(HPC):
                                w_ = wpp.tile([P, C], BF16, tag=f"wp{hh}",
                                              name=f"wp{hh}")
                                nc.sync.dma_start(
                                    w_[:], wp[hh * P:(hh + 1) * P, :])
                                wp_t.append(w_)
                        for t in range(TT):
                            emit_C_t(h, t, y_ref[h])
                            if h + 1 < HPC:
                                emit_B_t(h + 1, t)

                # ---- phase D: output projection ----
                with tc.tile_pool(name="psd", bufs=1, space="PSUM") as psd:
                    for m in range(T // P):
                        msl = slice(m * P, (m + 1) * P)
                        pso = [
                            psd.tile([P, NT], F32, tag=f"pso{n}", bufs=2,
                                     name=f"pso{n}")
                            for n in range(4)
                        ]
                        for hh in range(HPC):
                            lhsT = y_ref[hh][:, msl]
                            for n in range(4):
                                _mm(pso[n][:], lhsT,
                                    wp_t[hh][:, n * NT:(n + 1) * NT],
                                    start=(hh == 0), stop=(hh == HPC - 1))
                        for half in range(2):
                            ot = wpp.tile([P, 2 * NT], F32, tag="ot",
                                          bufs=2, name="ot")
                            for k in range(2):
                                n = 2 * half + k
                                nc.scalar.copy(ot[:, k * NT:(k + 1) * NT],
                                               pso[n][:])
                            nc.gpsimd.dma_start(
                                out[msl, half * 2 * NT:(half + 1) * 2 * NT],
                                ot[:])

    nc.finalize()
    return nc


def prep_inputs(x, w_attn, b_attn, w_proj, b_proj):
    """Build the 8 per-core input maps from full inputs (bf16)."""
    x = np.asarray(x, dtype=np.float32)
    w_attn = np.asarray(w_attn, dtype=np.float32)
    b_attn = np.asarray(b_attn, dtype=np.float32)
    w_proj = np.asarray(w_proj, dtype=np.float32)

    scale = np.float32(1.0 / np.sqrt(D))

    inv_freq = 1.0 / (ROPE_BASE ** (np.arange(0, D, 2, dtype=np.float32) / D))
    tpos = np.arange(T, dtype=np.float32)
    ang = np.outer(tpos, inv_freq)  # [T, 64]
    cos_t, sin_t = np.cos(ang).T, np.sin(ang).T  # [64, T]
    cs = np.ascontiguousarray(
        np.concatenate([cos_t, cos_t], axis=0)).astype(BF_NP)
    sw = np.ascontiguousarray(
        np.concatenate([-sin_t, sin_t], axis=0)).astype(BF_NP)

    qq = np.arange(P)
    kk = np.arange(P)[:, None]
    tri = np.ascontiguousarray(
        (qq[None, :] >= kk).astype(BF_NP))  # [128,128] causal triangle
    onesf = np.ones((P, P), dtype=np.float32)

    in_maps = []
    bvps = []
    for core in range(8):
        b = core // 2
        hg = core % 2
        heads = range(hg * HPC, (hg + 1) * HPC)
        qcols = np.concatenate([np.arange(h * D, (h + 1) * D) for h in heads])
        kcols = qcols + C
        vcols = qcols + 2 * C

        # per-head [C, 2P] blocks stacked: rows h*C..(h+1)*C
        wqk_blocks = []
        bqk_s = np.zeros((P, 2 * HPC), dtype=np.float32)
        for i, h in enumerate(heads):
            qc = np.arange(h * D, (h + 1) * D)
            kc = qc + C
            wqk_blocks.append(np.concatenate(
                [w_attn[:, qc] * scale, w_attn[:, kc]], axis=1))
            bqk_s[:, 2 * i] = b_attn[qc] * scale
            bqk_s[:, 2 * i + 1] = b_attn[kc]
        wqk_s = np.ascontiguousarray(
            np.concatenate(wqk_blocks, axis=0)).astype(BF_NP)

        wv_s = np.ascontiguousarray(w_attn[:, vcols]).astype(BF_NP)
        wp_s = np.ascontiguousarray(w_proj[qcols, :]).astype(BF_NP)
        xt_s = np.ascontiguousarray(x[b].T).astype(BF_NP)
        bvps.append(b_attn[vcols] @ w_proj[qcols, :])  # fp32 host partial

        in_maps.append({
            "xt": xt_s, "wqk": wqk_s, "bqk": np.ascontiguousarray(bqk_s),
            "wv": wv_s, "cs": cs, "sw": sw, "tri": tri, "onesf": onesf,
            "wp": wp_s,
        })
    return in_maps, bvps


def _get_program():
    if "nc" not in _CACHE:
        _CACHE["nc"] = build_program()
    return _CACHE["nc"]


def _postprocess(outs, bvps, b_proj):
    b_proj = np.asarray(b_proj, dtype=np.float32)
    return np.stack([
        outs[2 * b] + outs[2 * b + 1]
        + (bvps[2 * b] + bvps[2 * b + 1] + b_proj)[None, :]
        for b in range(B)
    ]).astype(np.float32)


def _run(inputs, trace=False):
    from concourse.bass_utils import run_bass_kernel_spmd

    nc = _get_program()
    in_maps, bvps = prep_inputs(
        inputs["x"], inputs["w_attn"], inputs["b_attn"],
        inputs["w_proj"], inputs["b_proj"],
    )
    res = run_bass_kernel_spmd(nc, in_maps, core_ids=list(range(8)),
                               trace=trace)
    full = _postprocess([r["out"] for r in res.results], bvps,
                        inputs["b_proj"])
    return full, res


def kernel(**inputs):
    full, _ = _run(inputs, trace=False)
    return full


if __name__ == "__main__":
    _get_program()
    print("built ok")


# revision 22
# speedup vs baseline: 1.4764x; 1.4764x over previous
"""Trainium2 Bass kernel for causal self-attention (B=4, T=2048, C=2048, H=16).

Sharding: 8 cores = 4 batches x 2 head-groups (8 heads each).
v2 architecture (all-bf16 dataflow, fused QK-projection + SDPA pipeline):
  A) v = x @ Wv (bf16)            -> DRAM spill [T, 1024] bf16
  B+C fused, per head h (software pipelined, slot h = C(h) || B(h+1)):
     B-step: qT/kT = RoPE(Wqk^T x^T + b)  -> SBUF resident bf16 [128, T]
     C-step: per (t, j): scores matmul -> exp (ACT, bf16) -> tri mask ->
             PV matmul + denominator matmul (ones lhsT) accumulate in PSUM;
             per t: rden = reciprocal_approx_fast(den); y = psy * rden (bf16)
  D) partial_out = y^T @ wp  -> [T, C] fp32 partial
Host sums core pairs per batch, adds b_proj and the folded v-bias term
(bv @ wp) per core.
"""

import sys

import numpy as np

sys.path.insert(0, "/opt/trn_rl_repo")

import ml_dtypes  # noqa: E402

import concourse.bass as bass  # noqa: E402,F401
import concourse.mybir as mybir  # noqa: E402
import concourse.tile as tile  # noqa: E402
from concourse import bacc  # noqa: E402

F32 = mybir.dt.float32
BF16 = mybir.dt.bfloat16
AF = mybir.ActivationFunctionType
BF_NP = ml_dtypes.bfloat16

B, T, C = 4, 2048, 2048
H, D = 16, 128
HPC = 8            # heads per core
P = 128
NT = 512           # matmul moving free dim
TT = T // NT       # 4 token tiles
CC = C // P        # 16 contraction chunks over C
HD = D // 2        # 64 (rope half)
ROPE_BASE = 10000.0
LAG = 2            # score->PV software pipeline depth (in j blocks)

_CACHE = {}


def build_program():
    nc = bacc.Bacc(name="csa_tp2")

    xt = nc.dram_tensor("xt", (C, T), BF16, kind="ExternalInput")
    wv = nc.dram_tensor("wv", (C, HPC * D), BF16, kind="ExternalInput")
    # per-head blocks of [C, 2P] (q columns scaled, then k columns)
    wqk = nc.dram_tensor("wqk", (HPC * C, 2 * P), BF16, kind="ExternalInput")
    bqk = nc.dram_tensor("bqk", (P, 2 * HPC), F32, kind="ExternalInput")
    cs = nc.dram_tensor("cs", (P, T), BF16, kind="ExternalInput")
    sw = nc.dram_tensor("sw", (P, T), BF16, kind="ExternalInput")
    tri = nc.dram_tensor("tri", (P, P), BF16, kind="ExternalInput")
    onesm = nc.dram_tensor("onesm", (P, P), BF16, kind="ExternalInput")
    onesr = nc.dram_tensor("onesr", (P, P), mybir.dt.float32r,
                           kind="ExternalInput")
    wp = nc.dram_tensor("wp", (HPC * D, C), BF16, kind="ExternalInput")
    out = nc.dram_tensor("out", (T, C), F32, kind="ExternalOutput")

    v_spill = nc.dram_tensor("v_spill", (T, HPC * D), BF16, kind="Internal")

    def _mm(o, lhsT, rhs, **kw):
        nc.tensor.matmul(o, lhsT, rhs, **kw)

    with tile.TileContext(nc) as tc:
        with (
            tc.tile_pool(name="tabs", bufs=1) as tabs,
            tc.tile_pool(name="work", bufs=1) as work,
            tc.tile_pool(name="ypool", bufs=1) as ypool,
            tc.tile_pool(name="xtp", bufs=1) as xtp,
        ):
            # ---- tables + first wqk prefetches (gpsimd queue) ----
            cs_t = tabs.tile([P, T], BF16, tag="cs", name="cs")
            nc.gpsimd.dma_start(cs_t[:], cs[:])
            sw_t = tabs.tile([P, T], BF16, tag="sw", name="sw")
            nc.gpsimd.dma_start(sw_t[:], sw[:])
            tri_t = tabs.tile([P, P], BF16, tag="tri", name="tri")
            nc.gpsimd.dma_start(tri_t[:], tri[:])
            ones_t = tabs.tile([P, P], BF16, tag="ones", name="ones")
            nc.gpsimd.dma_start(ones_t[:], onesm[:])
            onesr_t = tabs.tile([P, P], mybir.dt.float32r, tag="onesr",
                                name="onesr")
            nc.gpsimd.dma_start(onesr_t[:], onesr[:])
            bqk_t = tabs.tile([P, 2 * HPC], F32, tag="bqk", name="bqk")
            nc.gpsimd.dma_start(bqk_t[:], bqk[:])

            wq_ref = {}

            def prefetch_wqk(h):
                w_ = work.tile([P, CC, 2 * P], BF16, tag=f"wq{h % 2}",
                               name=f"wq{h % 2}")
                nc.gpsimd.dma_start(
                    w_[:],
                    wqk[h * C:(h + 1) * C, :].rearrange(
                        "(c p) f -> p c f", p=P),
                )
                wq_ref[h] = w_

            vh_ref = {}

            def prefetch_vh(h):
                v_ = work.tile([P, CC, P], BF16, tag=f"vh{h % 2}",
                               name=f"vh{h % 2}")
                nc.sync.dma_start(
                    v_[:],
                    v_spill[:, h * D:(h + 1) * D].rearrange(
                        "(j p) d -> p j d", p=P),
                )
                vh_ref[h] = v_

            prefetch_wqk(0)
            prefetch_wqk(1)

            # ---- phase A: V projection (bf16), v -> DRAM spill ----
            xtt = [[None] * TT for _ in range(CC)]
            wvh = [[None] * CC for _ in range(2)]
            with (
                tc.tile_pool(name="wvp", bufs=1) as wvp,
                tc.tile_pool(name="psa", bufs=1, space="PSUM") as psa,
            ):
                # interleave first xt token-slice with wv so the first
                # matmul chain's deps land early (different queues)
                def load_xt(t):
                    for c in range(CC):
                        x_ = xtp.tile([P, NT], BF16, tag=f"x{c}_{t}",
                                      name=f"x{c}_{t}")
                        nc.sync.dma_start(
                            x_[:], xt[c * P:(c + 1) * P, t * NT:(t + 1) * NT])
                        xtt[c][t] = x_

                for c in range(CC):
                    for n in range(2):
                        w_ = wvp.tile([P, NT], BF16, tag=f"wv{n}_{c}",
                                      name=f"wv{n}_{c}")
                        nc.scalar.dma_start(
                            w_[:], wv[c * P:(c + 1) * P,
                                      n * NT:(n + 1) * NT])
                        wvh[n][c] = w_
                load_xt(0)
                load_xt(1)

                for t in range(TT):
                    for m in range(4):
                        mtok = 4 * t + m
                        msl = slice(m * P, (m + 1) * P)
                        ps0 = psa.tile([P, NT], F32, tag="psa0", bufs=2,
                                       name="psa0")
                        ps1 = psa.tile([P, NT], F32, tag="psa1", bufs=2,
                                       name="psa1")
                        for c in range(CC):
                            _mm(ps0[:], xtt[c][t][:, msl], wvh[0][c][:],
                                start=(c == 0), stop=(c == CC - 1))
                            _mm(ps1[:], xtt[c][t][:, msl], wvh[1][c][:],
                                start=(c == 0), stop=(c == CC - 1))
                        vt = work.tile([P, 2 * NT], BF16, tag="vt", bufs=2,
                                       name="vt")
                        nc.scalar.copy(vt[:, 0:NT], ps0[:])
                        nc.scalar.copy(vt[:, NT:2 * NT], ps1[:])
                        nc.gpsimd.dma_start(
                            v_spill[mtok * P:(mtok + 1) * P, :], vt[:])
                    if t + 2 < TT:
                        load_xt(t + 2)

            prefetch_vh(0)
            prefetch_vh(1)

            # ---- fused B+C pipeline ----
            with tc.tile_pool(name="wpp", bufs=1) as wpp:
                with tc.tile_pool(name="psbc", bufs=1, space="PSUM") as psbc:
                    qs_ref = {}
                    ks_ref = {}

                    def emit_B_t(h, t):
                        tsl = slice(t * NT, (t + 1) * NT)
                        wq_h = wq_ref[h]
                        for f in range(2):
                            pst = psbc.tile([P, NT], F32, tag="pst", bufs=2,
                                            name="pst")
                            for c in range(CC):
                                _mm(pst[:], wq_h[:, c, f * P:(f + 1) * P],
                                    xtt[c][t][:],
                                    start=(c == 0), stop=(c == CC - 1))
                            bcol = bqk_t[:, 2 * h + f:2 * h + f + 1]
                            rr = work.tile([P, NT], BF16, tag="rr", bufs=2,
                                           name="rr")
                            nc.scalar.activation(
                                rr[:], pst[:], AF.Identity, bias=bcol)
                            rw = work.tile([P, NT], BF16, tag="rw", bufs=2,
                                           name="rw")
                            nc.scalar.activation(
                                rw[0:HD, :], pst[HD:P, :], AF.Identity,
                                bias=bqk_t[HD:P, 2 * h + f:2 * h + f + 1])
                            nc.scalar.activation(
                                rw[HD:P, :], pst[0:HD, :], AF.Identity,
                                bias=bqk_t[0:HD, 2 * h + f:2 * h + f + 1])
                            dst = (qs_ref if f == 0 else ks_ref)[h][:, tsl]
                            tmp = work.tile([P, NT], BF16, tag="tmp", bufs=2,
                                            name="tmp")
                            nc.vector.tensor_mul(dst, rr[:], cs_t[:, tsl])
                            nc.vector.tensor_mul(tmp[:], rw[:], sw_t[:, tsl])
                            nc.vector.tensor_add(dst, dst, tmp[:])

                    def new_qk(h):
                        qs_ref[h] = work.tile([P, T], BF16, tag=f"qs{h % 2}",
                                              name=f"qs{h % 2}")
                        ks_ref[h] = work.tile([P, T], BF16, tag=f"ks{h % 2}",
                                              name=f"ks{h % 2}")

                    def emit_C_t(h, t, y_t):
                        tsl = slice(t * NT, (t + 1) * NT)
                        qs, ks, vh_ = qs_ref[h], ks_ref[h], vh_ref[h]
                        njs = 4 * t + 4
                        psy = psbc.tile([P, NT], F32, tag="psy", bufs=1,
                                        name="psy")
                        den = psbc.tile([P, NT], F32, tag="psd", bufs=1,
                                        name="psd")
                        # denominator: diag blocks + 1/3 of full blocks on
                        # PE (ones matmul); the rest accumulated on DVE in
                        # fp32 (exact)
                        pe_js = [j for j in range(njs)
                                 if j >= 4 * t or j % 3 == 0]
                        dve_js = [j for j in range(njs) if j not in pe_js]
                        p_sum = work.tile([P, NT], mybir.dt.float32r,
                                          tag="p_sum", bufs=1, name="p_sum")
                        saved = [None] * njs
                        for jj in range(njs + LAG):
                            if jj < njs:
                                j = jj
                                off = (j - 4 * t) * P if j >= 4 * t else 0
                                pss = psbc.tile([P, NT], F32, tag="pss",
                                                bufs=4, name="pss")
                                _mm(pss[:, off:], ks[:, j * P:(j + 1) * P],
                                    qs[:, t * NT + off:(t + 1) * NT],
                                    start=True, stop=True)
                                p = work.tile([P, NT], BF16, tag="p", bufs=4,
                                              name="p")
                                nc.scalar.activation(
                                    p[:, off:], pss[:, off:], AF.Exp)
                                if j >= 4 * t:
                                    nc.vector.tensor_mul(
                                        p[:, off:off + P],
                                        p[:, off:off + P], tri_t[:])
                                if j in dve_js:
                                    if j == dve_js[0]:
                                        nc.vector.tensor_copy(p_sum[:], p[:])
                                    else:
                                        nc.vector.tensor_add(
                                            p_sum[:], p_sum[:], p[:])
                                saved[j] = (p, off)
                            if jj >= LAG:
                                j2 = jj - LAG
                                p2, off2 = saved[j2]
                                _mm(psy[:, off2:], vh_[:, j2, :],
                                    p2[:, off2:],
                                    start=(j2 == 0), stop=(j2 == njs - 1))
                                if j2 in pe_js:
                                    _mm(den[:, off2:], ones_t[:],
                                        p2[:, off2:],
                                        start=(j2 == pe_js[0]),
                                        stop=(j2 == pe_js[-1]
                                              and not dve_js))
                        rden = work.tile([P, NT], F32, tag="rden", bufs=2,
                                         name="rden")
                        if dve_js:
                            # partition-reduce the DVE partial sums into the
                            # same den accumulation group (f32r rhs, full
                            # rate at N=512)
                            _mm(den[:], onesr_t[:], p_sum[:],
                                start=False, stop=True)
                        nc.vector.reciprocal_approx_fast(
                            out=rden[:], in_=den[:])
                        nc.vector.tensor_mul(y_t[:, tsl], psy[:], rden[:])

                    # prologue: B(0)
                    new_qk(0)
                    for t in range(TT):
                        emit_B_t(0, t)

                    y_ref = {}
                    wp_t = []
                    for h in range(HPC):
                        y_ref[h] = ypool.tile([P, T], BF16, tag=f"y{h}",
                                              name=f"y{h}")
                        if h + 2 < HPC:
                            prefetch_wqk(h + 2)
                            prefetch_vh(h + 2)
                        if h + 1 < HPC:
                            new_qk(h + 1)
                        if h == HPC - 1:
                            # wp loads (vector queue) ahead of phase D
                            for hh in range(HPC):
                                w_ = wpp.tile([P, C], BF16, tag=f"wp{hh}",
                                              name=f"wp{hh}")
                                nc.sync.dma_start(
                                    w_[:], wp[hh * P:(hh + 1) * P, :])
                                wp_t.append(w_)
                        for t in range(TT):
                            emit_C_t(h, t, y_ref[h])
                            if h + 1 < HPC:
                                emit_B_t(h + 1, t)

                # ---- phase D: output projection ----
                with tc.tile_pool(name="psd", bufs=1, space="PSUM") as psd:
                    for m in range(T // P):
                        msl = slice(m * P, (m + 1) * P)
                        pso = [
                            psd.tile([P, NT], F32, tag=f"pso{n}", bufs=2,
                                     name=f"pso{n}")
                            for n in range(4)
                        ]
                        for hh in range(HPC):
                            lhsT = y_ref[hh][:, msl]
                            for n in range(4):
                                _mm(pso[n][:], lhsT,
                                    wp_t[hh][:, n * NT:(n + 1) * NT],
                                    start=(hh == 0), stop=(hh == HPC - 1))
                        for half in range(2):
                            ot = wpp.tile([P, 2 * NT], F32, tag="ot",
                                          bufs=2, name="ot")
                            for k in range(2):
                                n = 2 * half + k
                                nc.scalar.copy(ot[:, k * NT:(k + 1) * NT],
                                               pso[n][:])
                            nc.gpsimd.dma_start(
                                out[msl, half * 2 * NT:(half + 1) * 2 * NT],
                                ot[:])

    nc.finalize()
    return nc


def prep_inputs(x, w_attn, b_attn, w_proj, b_proj):
    """Build the 8 per-core input maps from full inputs (bf16)."""
    x = np.asarray(x, dtype=np.float32)
    w_attn = np.asarray(w_attn, dtype=np.float32)
    b_attn = np.asarray(b_attn, dtype=np.float32)
    w_proj = np.asarray(w_proj, dtype=np.float32)

    scale = np.float32(1.0 / np.sqrt(D))

    inv_freq = 1.0 / (ROPE_BASE ** (np.arange(0, D, 2, dtype=np.float32) / D))
    tpos = np.arange(T, dtype=np.float32)
    ang = np.outer(tpos, inv_freq)  # [T, 64]
    cos_t, sin_t = np.cos(ang).T, np.sin(ang).T  # [64, T]
    cs = np.ascontiguousarray(
        np.concatenate([cos_t, cos_t], axis=0)).astype(BF_NP)
    sw = np.ascontiguousarray(
        np.concatenate([-sin_t, sin_t], axis=0)).astype(BF_NP)

    qq = np.arange(P)
    kk = np.arange(P)[:, None]
    tri = np.ascontiguousarray(
        (qq[None, :] >= kk).astype(BF_NP))  # [128,128] causal triangle
    onesm = np.ones((P, P), dtype=BF_NP)
    onesr = np.ones((P, P), dtype=np.float32)

    in_maps = []
    bvps = []
    for core in range(8):
        b = core // 2
        hg = core % 2
        heads = range(hg * HPC, (hg + 1) * HPC)
        qcols = np.concatenate([np.arange(h * D, (h + 1) * D) for h in heads])
        kcols = qcols + C
        vcols = qcols + 2 * C

        # per-head [C, 2P] blocks stacked: rows h*C..(h+1)*C
        wqk_blocks = []
        bqk_s = np.zeros((P, 2 * HPC), dtype=np.float32)
        for i, h in enumerate(heads):
            qc = np.arange(h * D, (h + 1) * D)
            kc = qc + C
            wqk_blocks.append(np.concatenate(
                [w_attn[:, qc] * scale, w_attn[:, kc]], axis=1))
            bqk_s[:, 2 * i] = b_attn[qc] * scale
            bqk_s[:, 2 * i + 1] = b_attn[kc]
        wqk_s = np.ascontiguousarray(
            np.concatenate(wqk_blocks, axis=0)).astype(BF_NP)

        wv_s = np.ascontiguousarray(w_attn[:, vcols]).astype(BF_NP)
        wp_s = np.ascontiguousarray(w_proj[qcols, :]).astype(BF_NP)
        xt_s = np.ascontiguousarray(x[b].T).astype(BF_NP)
        bvps.append(b_attn[vcols] @ w_proj[qcols, :])  # fp32 host partial

        in_maps.append({
            "xt": xt_s, "wqk": wqk_s, "bqk": np.ascontiguousarray(bqk_s),
            "wv": wv_s, "cs": cs, "sw": sw, "tri": tri, "onesm": onesm,
            "onesr": onesr, "wp": wp_s,
        })
    return in_maps, bvps


def _get_program():
    if "nc" not in _CACHE:
        _CACHE["nc"] = build_program()
    return _CACHE["nc"]


def _postprocess(outs, bvps, b_proj):
    b_proj = np.asarray(b_proj, dtype=np.float32)
    return np.stack([
        outs[2 * b] + outs[2 * b + 1]
        + (bvps[2 * b] + bvps[2 * b + 1] + b_proj)[None, :]
        for b in range(B)
    ]).astype(np.float32)


def _run(inputs, trace=False):
    from concourse.bass_utils import run_bass_kernel_spmd

    nc = _get_program()
    in_maps, bvps = prep_inputs(
        inputs["x"], inputs["w_attn"], inputs["b_attn"],
        inputs["w_proj"], inputs["b_proj"],
    )
    res = run_bass_kernel_spmd(nc, in_maps, core_ids=list(range(8)),
                               trace=trace)
    full = _postprocess([r["out"] for r in res.results], bvps,
                        inputs["b_proj"])
    return full, res


def kernel(**inputs):
    full, _ = _run(inputs, trace=False)
    return full


if __name__ == "__main__":
    _get_program()
    print("built ok")
